# revision 1
# baseline (speedup 1.0000x reference)
"""LorentzTransformer Trainium2 kernel.

Full inputs in, full output out. Sharding: 8 cores = 2 batches x 4 head
groups (4 heads / 256 channels each). Host pre-transposes x and the weight
shards so every on-chip matmul has its contraction dim on partitions.

Per-core pipeline (fp16 PE datapath, fp32 PSUM accumulation):
  QT/KT = W-proj of x (head channels on partitions, seq on free)
  V     = natural-layout proj, augmented with a ones column (softmax denom)
  Qeff  = Q * (1 - 2*alpha*sf*m) / sqrt(dh); sf via PE partition-sum matmuls
  scoresT[k,q], head pairs row-packed on the PE -> exp on ACT -> causal via
  block skipping + one triangular 0/1 tile, N shrunk to visible columns
  AV + denom in one PSUM accumulation group; normalize via fast reciprocal +
  partition_broadcast (PE ones-matmul broadcast for the tail group)
  partial out = A @ Wo_shard.T, emitted last as PE gap-filler; host sums the
  4 head-group partials per batch
"""

import numpy as np

from concourse import bacc
import concourse.tile as tile
import concourse.mybir as mybir
from concourse.bass_utils import run_bass_kernel_spmd

B, L, D, H = 2, 1024, 1024, 16
DH = D // H  # 64
ALPHA = 0.25
SCALE = float(np.sqrt(DH))  # 8.0
HPC = 4          # heads per core
DPC = HPC * DH   # 256 channels per core
N_CORES = 8
P = 128
NQC = L // 512   # q chunks of 512
NKT = L // P     # k tiles of 128

FP = mybir.dt.float32
# PE compute dtype: fp16 runs the PE at full rate on the normal datapath
# (the HAM clock gate ignores fp32r matmuls and throttles to 1.2 GHz), gets
# fast-weight-load, and keeps 11 mantissa bits. PSUM accumulation is fp32.
FPC = mybir.dt.float16
NPC = np.float16




def _build_program():
    nc = bacc.Bacc("TRN2", target_bir_lowering=False)

    xT = nc.dram_tensor("xT", [D, L], FPC, kind="ExternalInput")
    wqT = nc.dram_tensor("wqT", [D, DPC], FPC, kind="ExternalInput")
    wkT = nc.dram_tensor("wkT", [D, DPC], FPC, kind="ExternalInput")
    wvT = nc.dram_tensor("wvT", [D, DPC], FPC, kind="ExternalInput")
    woT = nc.dram_tensor("woT", [DPC, D], FPC, kind="ExternalInput")
    normblk = nc.dram_tensor("normblk", [P, 2, 4], FPC, kind="ExternalInput")
    sprime = nc.dram_tensor("sprime", [2, 2, P], FPC, kind="ExternalInput")
    maskT = nc.dram_tensor("maskT", [P, P], FPC, kind="ExternalInput")
    out = nc.dram_tensor("out", [L, D], FP, kind="ExternalOutput")

    with tile.TileContext(nc) as tc:
        with (
            tc.tile_pool(name="persist", bufs=1) as persist,
            tc.tile_pool(name="work", bufs=3) as work,
            tc.tile_pool(name="expp", bufs=8) as expp,
            tc.tile_pool(name="sm", bufs=6) as smp,
            tc.tile_pool(name="ost", bufs=3) as ost,
            tc.tile_pool(name="psA", bufs=2, space="PSUM") as psA,
            tc.tile_pool(name="psS", bufs=3, space="PSUM") as psS,
            tc.tile_pool(name="psV", bufs=3, space="PSUM") as psV,
        ):
            # ---- load inputs ----
            xT_r = xT.rearrange("(o p) l -> p o l", p=P)
            xT_sb = [persist.tile([P, L], FPC, tag=f"xT{k}", name=f"xT{k}") for k in range(D // P)]
            wq_r = wqT.rearrange("(o p) n -> p o n", p=P)
            wq_sb = [persist.tile([P, DPC], FPC, tag=f"wq{k}", name=f"wq{k}") for k in range(D // P)]
            wk_r = wkT.rearrange("(o p) n -> p o n", p=P)
            wk_sb = [persist.tile([P, DPC], FPC, tag=f"wk{k}", name=f"wk{k}") for k in range(D // P)]
            wv_r = wvT.rearrange("(o p) n -> p o n", p=P)
            wv_sb = [persist.tile([P, DPC], FPC, tag=f"wv{k}", name=f"wv{k}") for k in range(D // P)]
            nb_sb = persist.tile([P, 2, 4], FPC, tag="nb")
            nc.sync.dma_start(nb_sb[:], normblk[:])
            sp_sb = persist.tile([2, 2, P], FPC, tag="sp")
            nc.sync.dma_start(sp_sb[:], sprime[:])
            mk_sb = persist.tile([P, P], FPC, tag="mk")
            nc.sync.dma_start(mk_sb[:], maskT[:])
            for k in range(D // P):
                nc.sync.dma_start(xT_sb[k][:], xT_r[:, k])
                nc.sync.dma_start(wq_sb[k][:], wq_r[:, k])
            for k in range(D // P):
                nc.sync.dma_start(wk_sb[k][:], wk_r[:, k])
                nc.sync.dma_start(wv_sb[k][:], wv_r[:, k])
            wo_sb = persist.tile([P, DPC // P, D], FPC, tag="wo")
            nc.sync.dma_start(wo_sb[:], woT.rearrange("(o p) n -> p o n", p=P))

            qT_sb = [persist.tile([P, L], FPC, tag=f"qT{t}", name=f"qT{t}") for t in range(2)]
            kT_sb = [persist.tile([P, L], FPC, tag=f"kT{t}", name=f"kT{t}") for t in range(2)]
            # V' with ones column per (ktile, head)
            v_sb = persist.tile([P, NKT, HPC, DH + 1], FPC, tag="v")
            onecol = persist.tile([P, 1], FP, tag="onecol")
            nc.vector.memset(onecol[:], 1.0)
            nc.vector.tensor_copy(
                v_sb[:, :, :, DH : DH + 1],
                onecol.to_broadcast([P, NKT, HPC, 1]),
            )

            ones_row = persist.tile([1, DH], FPC, tag="ones_row")
            nc.vector.memset(ones_row[:], 1.0)

            aT_sb = [
                [
                    persist.tile([P, 512], FPC, tag=f"aT{t}_{qc}", name=f"aT{t}_{qc}")
                    for qc in range(NQC)
                ]
                for t in range(2)
            ]

            # ---- QT proj (t-tile at a time) + Lorentz factor, then KT, V ----
            def proj(w_sb, dst, t):
                for qc in range(NQC):
                    ps = psA.tile([P, 512], FP, tag="psA", name="proj")
                    for k in range(D // P):
                        nc.tensor.matmul(
                            ps[:],
                            w_sb[k][:, t * P : (t + 1) * P],
                            xT_sb[k][:, qc * 512 : (qc + 1) * 512],
                            start=(k == 0),
                            stop=(k == D // P - 1),
                        )
                    nc.vector.tensor_copy(dst[t][:, qc * 512 : (qc + 1) * 512], ps[:])

            def lorentz(t):
                # QeffT = QT * (0.125 - 0.0625*sf*m), sf = |Q|/|Qt| per (head,q)
                sq = work.tile([P, L], FPC, tag="sq")
                nc.vector.tensor_mul(sq[:], qT_sb[t][:], qT_sb[t][:])
                sf = work.tile([2, L], FPC, tag="sf")
                for qc in range(NQC):
                    nps = psS.tile([P, 512], FP, tag="psS", name="nps")
                    nc.tensor.matmul(
                        nps[:2, :],
                        nb_sb[:, t, 0:2],
                        sq[:, qc * 512 : (qc + 1) * 512],
                        start=True,
                        stop=True,
                    )
                    nqs = psS.tile([P, 512], FP, tag="psS", name="nqs")
                    nc.tensor.matmul(
                        nqs[:2, :],
                        nb_sb[:, t, 2:4],
                        sq[:, qc * 512 : (qc + 1) * 512],
                        start=True,
                        stop=True,
                    )
                    brcp = smp.tile([2, 512], FP, tag="brcp")
                    nc.vector.reciprocal_approx_fast(brcp[:], nqs[0:2, :])
                    rat = smp.tile([2, 512], FP, tag="rat")
                    nc.vector.tensor_mul(rat[:], nps[0:2, :], brcp[:])
                    nc.scalar.activation(
                        sf[:, qc * 512 : (qc + 1) * 512],
                        rat[:],
                        mybir.ActivationFunctionType.Sqrt,
                    )
                for qc in range(NQC):
                    gps = psS.tile([P, 512], FP, tag="psS", name="gps")
                    nc.tensor.matmul(
                        gps[:],
                        sp_sb[:, t, :],
                        sf[:, qc * 512 : (qc + 1) * 512],
                        start=True,
                        stop=True,
                    )
                    fp_sb = smp.tile([P, 512], FPC, tag="fp")
                    nc.vector.tensor_scalar_add(fp_sb[:], gps[:], 1.0 / SCALE)
                    nc.vector.tensor_mul(
                        qT_sb[t][:, qc * 512 : (qc + 1) * 512],
                        qT_sb[t][:, qc * 512 : (qc + 1) * 512],
                        fp_sb[:],
                    )

            for t in range(2):
                proj(wq_sb, qT_sb, t)
                lorentz(t)
            for t in range(2):
                proj(wk_sb, kT_sb, t)

            # ---- V natural layout: out[l, dv], packed into V' ----
            for lt in range(NKT):
                ps = psA.tile([P, 512], FP, tag="psA", name="vproj")
                for k in range(D // P):
                    nc.tensor.matmul(
                        ps[:, :DPC],
                        xT_sb[k][:, lt * P : (lt + 1) * P],
                        wv_sb[k][:, :],
                        start=(k == 0),
                        stop=(k == D // P - 1),
                    )
                nc.vector.tensor_copy(
                    v_sb[:, lt, :, :DH],
                    ps[:, :DPC].rearrange("p (h d) -> p h d", h=HPC),
                )

            # ---- attention (head pairs row-packed) interleaved with Wo ----
            def attn_group(t, qc):
                avs = [
                    psV.tile([DH + 1, 512], FP, tag="psV", name=f"av{hl}")
                    for hl in range(2)
                ]
                nkt = 4 * qc + 4  # causal: k tiles 0..4qc+3
                for kt in range(nkt):
                    off = max(0, (kt - 4 * qc) * P)  # first visible q col
                    n = 512 - off
                    exs = []
                    for hl in range(2):
                        base = hl * DH
                        sc = psS.tile([P, 512], FP, tag="psS", name=f"sc{hl}")
                        nc.tensor.matmul(
                            sc[:, off:512],
                            kT_sb[t][base : base + DH, kt * P : (kt + 1) * P],
                            qT_sb[t][
                                base : base + DH,
                                qc * 512 + off : (qc + 1) * 512,
                            ],
                            start=True,
                            stop=True,
                            tile_position=(base, 0),
                        )
                        ex = expp.tile([P, 512], FPC, tag="ex", name=f"ex{hl}")
                        nc.scalar.activation(
                            ex[:, off:512],
                            sc[:, off:512],
                            mybir.ActivationFunctionType.Exp,
                        )
                        j = kt - 4 * qc
                        if j >= 0:  # diagonal block gets the triangular mask
                            nc.vector.tensor_mul(
                                ex[:, j * P : (j + 1) * P],
                                ex[:, j * P : (j + 1) * P],
                                mk_sb[:],
                            )
                        exs.append(ex)
                    for hl in range(2):
                        nc.tensor.matmul(
                            avs[hl][:, off:512],
                            v_sb[:, kt, 2 * t + hl, :],
                            exs[hl][:, off:512],
                            start=(kt == 0),
                            stop=(kt == nkt - 1),
                        )
                tail = t == 1 and qc == NQC - 1
                for hl in range(2):
                    base = hl * DH
                    # free the PSUM bank fast: copy numerator + denom out
                    avr = work.tile([DH, 512], FP, tag="avr", name="avr")
                    nc.vector.tensor_copy(avr[:], avs[hl][:DH, :])
                    den = smp.tile([1, 512], FP, tag="den")
                    nc.vector.tensor_copy(den[:], avs[hl][DH : DH + 1, :])
                    rc = smp.tile([1, 512], FP, tag="rc")
                    nc.vector.reciprocal_approx_fast(rc[:], den[:])
                    if tail:
                        # final group gates the last Wo burst: broadcast on the
                        # (idle) PE via a K=1 matmul instead of gpsimd
                        rc16 = smp.tile([1, 512], FPC, tag="rc16")
                        nc.vector.tensor_copy(rc16[:], rc[:])
                        bcp = psS.tile([DH, 512], FP, tag="psS", name="bcp")
                        nc.tensor.matmul(
                            bcp[:], ones_row[:], rc16[:], start=True, stop=True
                        )
                        nc.vector.tensor_mul(
                            aT_sb[t][qc][base : base + DH, :], avr[:], bcp[:]
                        )
                    else:
                        bc = smp.tile([DH, 512], FP, tag="bc")
                        nc.gpsimd.partition_broadcast(bc[:], rc[:], channels=DH)
                        nc.vector.tensor_mul(
                            aT_sb[t][qc][base : base + DH, :],
                            avr[:],
                            bc[:],
                        )

            def wo_tile(lt):
                qc = lt // 4
                for jc in range(NQC):
                    ps = psA.tile([P, 512], FP, tag="psA", name="wops")
                    for t2 in range(2):
                        nc.tensor.matmul(
                            ps[:],
                            aT_sb[t2][qc][:, (lt % 4) * P : (lt % 4 + 1) * P],
                            wo_sb[:, t2, jc * 512 : (jc + 1) * 512],
                            start=(t2 == 0),
                            stop=(t2 == 1),
                        )
                    oc = ost.tile([P, 512], FP, tag="oc")
                    nc.vector.tensor_copy(oc[:], ps[:])
                    nc.sync.dma_start(
                        out[lt * P : (lt + 1) * P, jc * 512 : (jc + 1) * 512], oc[:]
                    )

            for qc in range(NQC):
                for t in range(2):
                    attn_group(t, qc)
            for lt in range(NKT):
                wo_tile(lt)

    nc.compile()
    return nc


_NC = None


def _host_inputs(x, Wq, Wk, Wv, Wo, timelike_mask):
    m_full = np.asarray(timelike_mask).astype(np.float32)
    mt = np.tril(np.ones((P, P), dtype=np.float32)).T.copy()  # maskT[k,q]=1 iff k<=q
    in_maps = []
    for c in range(N_CORES):
        b, g = divmod(c, HPC)
        sl = slice(g * DPC, (g + 1) * DPC)
        m = m_full[sl]  # [256]
        nb = np.zeros((P, 2, 4), dtype=np.float32)
        sp = np.zeros((2, 2, P), dtype=np.float32)
        for t in range(2):
            m_t = m[t * P : (t + 1) * P]
            nb[0:DH, t, 0] = 1.0
            nb[DH:P, t, 1] = 1.0
            nb[0:DH, t, 2] = m_t[0:DH]
            nb[DH:P, t, 3] = m_t[DH:P]
            coef = -2.0 * ALPHA / SCALE  # -0.0625
            sp[0, t, 0:DH] = coef * m_t[0:DH]
            sp[1, t, DH:P] = coef * m_t[DH:P]
        in_maps.append(
            {
                "xT": np.ascontiguousarray(x[b].T).astype(NPC),
                "wqT": np.ascontiguousarray(Wq[sl, :].T).astype(NPC),
                "wkT": np.ascontiguousarray(Wk[sl, :].T).astype(NPC),
                "wvT": np.ascontiguousarray(Wv[sl, :].T).astype(NPC),
                "woT": np.ascontiguousarray(Wo[:, sl].T).astype(NPC),
                "normblk": nb.astype(NPC),
                "sprime": sp.astype(NPC),
                "maskT": mt.astype(NPC),
            }
        )
    return in_maps


def kernel(x, Wq, Wk, Wv, Wo, timelike_mask, attn_mask, _trace=False):
    global _NC
    if _NC is None:
        _NC = _build_program()
    nc = _NC

    x = np.asarray(x, dtype=np.float32)
    Wq, Wk, Wv, Wo = (np.asarray(w, dtype=np.float32) for w in (Wq, Wk, Wv, Wo))
    am = np.asarray(attn_mask, dtype=np.float32).reshape(L, L)
    causal = np.tril(np.ones((L, L), dtype=bool))
    assert np.array_equal(am, np.where(causal, 0.0, -1e9).astype(np.float32)), (
        "kernel hardcodes a causal additive mask"
    )

    in_maps = _host_inputs(x, Wq, Wk, Wv, Wo, timelike_mask)
    res = run_bass_kernel_spmd(
        nc, in_maps, core_ids=list(range(N_CORES)), trace=_trace
    )
    outp = np.stack(
        [
            sum(res.results[b * HPC + g]["out"] for g in range(HPC))
            for b in range(B)
        ]
    ).astype(np.float32)
    kernel.last_results = res
    return outp



# revision 13
# speedup vs baseline: 1.0670x; 1.0670x over previous
"""LorentzTransformer Trainium2 kernel.

Full inputs in, full output out. Sharding: 8 cores = 2 batches x 4 head
groups (4 heads / 256 channels each). Host pre-transposes x and the weight
shards so every on-chip matmul has its contraction dim on partitions.

Per-core pipeline (fp16 PE datapath, fp32 PSUM accumulation):
  QT/KT = W-proj of x (head channels on partitions, seq on free)
  V     = natural-layout proj, augmented with a ones column (softmax denom)
  Qeff  = Q * (0.125 - 0.0625*sf*m); sf via one M=4 PE partition-sum matmul,
  the +0.125 folded in as a third ones-row of the sprime matmul
  scoresT[k,q], head pairs row-packed on the PE -> exp on ACT -> causal via
  block skipping + one triangular 0/1 tile, N shrunk to visible columns
  AV + denom in one PSUM accumulation group; normalize straight out of PSUM
  (reciprocal + broadcast-multiply, no staging copies)
  partial out = A @ Wo_shard.T in fp16, interleaved into the second half of
  attention so the output DMA streams early; host sums the 4 head-group
  partials per batch

Scheduling: V-proj tail and K-proj(t1) are emitted as PE filler units inside
the attention t0 kt-loop (between the score matmuls and the AV matmuls), and
Wo(qc0) units fill attention t1 — the PE queue never drains while the ACT
engine computes exps, keeping the HAM clock gate warm.
"""

import numpy as np

from concourse import bacc
import concourse.tile as tile
import concourse.mybir as mybir
from concourse.bass_utils import run_bass_kernel_spmd

B, L, D, H = 2, 1024, 1024, 16
DH = D // H  # 64
ALPHA = 0.25
SCALE = float(np.sqrt(DH))  # 8.0
HPC = 4          # heads per core
DPC = HPC * DH   # 256 channels per core
N_CORES = 8
P = 128
KCH = D // P     # 8 contraction chunks
NQC = L // 512   # q chunks of 512
NKT = L // P     # k tiles of 128

FP = mybir.dt.float32
# PE compute dtype: fp16 runs the PE at full rate on the normal datapath
# (the HAM clock gate ignores fp32r matmuls and throttles to 1.2 GHz), gets
# fast-weight-load, and keeps 11 mantissa bits. PSUM accumulation is fp32.
FPC = mybir.dt.float16
NPC = np.float16


def _build_program(debug=False):
    nc = bacc.Bacc("TRN2", target_bir_lowering=False)

    xT = nc.dram_tensor("xT", [D, L], FPC, kind="ExternalInput")
    wqT = nc.dram_tensor("wqT", [D, DPC], FPC, kind="ExternalInput")
    wkT = nc.dram_tensor("wkT", [D, DPC], FPC, kind="ExternalInput")
    wvT = nc.dram_tensor("wvT", [D, DPC], FPC, kind="ExternalInput")
    woT = nc.dram_tensor("woT", [DPC, D], FPC, kind="ExternalInput")
    normblk = nc.dram_tensor("normblk", [P, 2, 34], FPC, kind="ExternalInput")
    sprime = nc.dram_tensor("sprime", [3, 2, P], FPC, kind="ExternalInput")
    maskT = nc.dram_tensor("maskT", [P, P], FPC, kind="ExternalInput")
    out = nc.dram_tensor("out", [L, D], FPC, kind="ExternalOutput")

    with tile.TileContext(nc) as tc:
        with (
            tc.tile_pool(name="persist", bufs=1) as persist,
            tc.tile_pool(name="work", bufs=2) as work,
            tc.tile_pool(name="expp", bufs=8) as expp,
            tc.tile_pool(name="sm", bufs=6) as smp,
            tc.tile_pool(name="ost", bufs=4) as ost,
            tc.tile_pool(name="psA", bufs=2, space="PSUM") as psA,
            tc.tile_pool(name="psS", bufs=3, space="PSUM") as psS,
            tc.tile_pool(name="psV", bufs=3, space="PSUM") as psV,
        ):
            # ---- persistent SBUF tiles ----
            xT_sb = persist.tile([P, KCH, L], FPC, tag="xT")
            wq_sb = persist.tile([P, KCH, DPC], FPC, tag="wq")
            wk_sb = persist.tile([P, KCH, DPC], FPC, tag="wk")
            wv_sb = persist.tile([P, KCH, DPC], FPC, tag="wv")
            wo_sb = persist.tile([P, DPC // P, D], FPC, tag="wo")
            nb_sb = persist.tile([P, 2, 34], FPC, tag="nb")
            sp_sb = persist.tile([3, 2, P], FPC, tag="sp")
            mk_sb = persist.tile([P, P], FPC, tag="mk")

            # ---- input DMA: batched, split across the two HWDGE queues,
            # ordered so the Q-projection dependencies land first ----
            nc.sync.dma_start(wq_sb[:], wqT.rearrange("(o p) n -> p o n", p=P))
            xT_r = xT.rearrange("(o p) l -> p o l", p=P)
            nc.sync.dma_start(xT_sb[:, 0:4, :], xT_r[:, 0:4])
            nc.sync.dma_start(xT_sb[:, 4:8, :], xT_r[:, 4:8])
            nc.sync.dma_start(wk_sb[:], wkT.rearrange("(o p) n -> p o n", p=P))
            nc.scalar.dma_start(nb_sb[:], normblk[:])
            nc.scalar.dma_start(sp_sb[:], sprime[:])
            nc.scalar.dma_start(wv_sb[:], wvT.rearrange("(o p) n -> p o n", p=P))
            nc.scalar.dma_start(mk_sb[:], maskT[:])
            nc.scalar.dma_start(wo_sb[:], woT.rearrange("(o p) n -> p o n", p=P))

            qT_sb = [persist.tile([P, L], FPC, tag=f"qT{t}", name=f"qT{t}") for t in range(2)]
            kT_sb = [persist.tile([P, L], FPC, tag=f"kT{t}", name=f"kT{t}") for t in range(2)]
            # V' with ones column per (ktile, head)
            v_sb = persist.tile([P, NKT, HPC, DH + 1], FPC, tag="v")
            onecol = persist.tile([P, 1], FP, tag="onecol")
            nc.vector.memset(onecol[:], 1.0)
            nc.vector.tensor_copy(
                v_sb[:, :, :, DH : DH + 1],
                onecol.to_broadcast([P, NKT, HPC, 1]),
            )

            ones_row = persist.tile([1, DH], FPC, tag="ones_row")
            nc.vector.memset(ones_row[:], 1.0)

            aT_sb = [
                [
                    persist.tile([P, 512], FPC, tag=f"aT{t}_{qc}", name=f"aT{t}_{qc}")
                    for qc in range(NQC)
                ]
                for t in range(2)
            ]

            # ---- projections ----
            def proj(w_sb, dst, t, qc):
                ps = psA.tile([P, 512], FP, tag="psA", name="proj")
                for k in range(KCH):
                    nc.tensor.matmul(
                        ps[:],
                        w_sb[:, k, t * P : (t + 1) * P],
                        xT_sb[:, k, qc * 512 : (qc + 1) * 512],
                        start=(k == 0),
                        stop=(k == KCH - 1),
                    )
                nc.vector.tensor_copy(dst[t][:, qc * 512 : (qc + 1) * 512], ps[:])

            def lorentz(t):
                # QeffT = QT * (0.125 - 0.0625*sf*m), sf = |Q|/|Qt| per (head,q)
                sq = work.tile([P, L], FPC, tag="sq")
                nc.scalar.square(sq[:], qT_sb[t][:])
                sf = work.tile([3, L], FPC, tag="sf")
                # base partition of an access must be 32-aligned: set the
                # whole 3-row tile to 1.0; rows 0:2 get overwritten below
                nc.vector.memset(sf[:], 1.0)
                for qc in range(NQC):
                    # one M=34 matmul: rows 0,1 = |Q|^2 per head, rows 32,33 =
                    # |Qt|^2 per head (operand bases must be 32-aligned)
                    nrm = psS.tile([P, 512], FP, tag="psS", name="nrm")
                    nc.tensor.matmul(
                        nrm[:34, :],
                        nb_sb[:, t, :],
                        sq[:, qc * 512 : (qc + 1) * 512],
                        start=True,
                        stop=True,
                    )
                    # custom-DVE ops silently drop a nonzero partition offset:
                    # stage rows 32:34 to a base-0 SBUF tile first
                    nq2 = smp.tile([2, 512], FP, tag="nq2")
                    nc.vector.tensor_copy(nq2[:], nrm[32:34, :])
                    brcp = smp.tile([2, 512], FP, tag="brcp")
                    nc.vector.reciprocal_approx_fast(brcp[:], nq2[:])
                    rat = smp.tile([2, 512], FP, tag="rat")
                    nc.vector.tensor_mul(rat[:], nrm[0:2, :], brcp[:])
                    nc.scalar.activation(
                        sf[0:2, qc * 512 : (qc + 1) * 512],
                        rat[:],
                        mybir.ActivationFunctionType.Sqrt,
                    )
                for qc in range(NQC):
                    gps = psS.tile([P, 512], FP, tag="psS", name="gps")
                    nc.tensor.matmul(
                        gps[:],
                        sp_sb[:, t, :],
                        sf[:, qc * 512 : (qc + 1) * 512],
                        start=True,
                        stop=True,
                    )
                    nc.vector.tensor_mul(
                        qT_sb[t][:, qc * 512 : (qc + 1) * 512],
                        qT_sb[t][:, qc * 512 : (qc + 1) * 512],
                        gps[:],
                    )

            # ---- V natural layout: out[l, dv], packed into V' ----
            def vproj(lt):
                ps = psA.tile([P, 512], FP, tag="psA", name="vproj")
                for k in range(KCH):
                    nc.tensor.matmul(
                        ps[:, :DPC],
                        xT_sb[:, k, lt * P : (lt + 1) * P],
                        wv_sb[:, k, :],
                        start=(k == 0),
                        stop=(k == KCH - 1),
                    )
                nc.vector.tensor_copy(
                    v_sb[:, lt, :, :DH],
                    ps[:, :DPC].rearrange("p (h d) -> p h d", h=HPC),
                )

            def kproj_half(t, qc, half, ps_box):
                # half 0: open the psA group, ks 0..3; half 1: ks 4..7 + copy
                if half == 0:
                    ps_box[0] = psA.tile([P, 512], FP, tag="psA", name="kproj")
                ps = ps_box[0]
                for k in range(half * 4, half * 4 + 4):
                    nc.tensor.matmul(
                        ps[:],
                        wk_sb[:, k, t * P : (t + 1) * P],
                        xT_sb[:, k, qc * 512 : (qc + 1) * 512],
                        start=(k == 0),
                        stop=(k == KCH - 1),
                    )
                if half == 1:
                    nc.vector.tensor_copy(
                        kT_sb[t][:, qc * 512 : (qc + 1) * 512], ps[:]
                    )

            # ---- Wo partial for one (lt, jc) output tile ----
            def wo_unit(lt, jc, oc_on_act):
                qc = lt // 4
                ps = psA.tile([P, 512], FP, tag="psA", name="wops")
                for t2 in range(2):
                    nc.tensor.matmul(
                        ps[:],
                        aT_sb[t2][qc][:, (lt % 4) * P : (lt % 4 + 1) * P],
                        wo_sb[:, t2, jc * 512 : (jc + 1) * 512],
                        start=(t2 == 0),
                        stop=(t2 == 1),
                    )
                oc = ost.tile([P, 512], FPC, tag="oc")
                if oc_on_act:
                    nc.scalar.activation(
                        oc[:], ps[:], mybir.ActivationFunctionType.Copy
                    )
                else:
                    nc.vector.tensor_copy(oc[:], ps[:])
                nc.sync.dma_start(
                    out[lt * P : (lt + 1) * P, jc * 512 : (jc + 1) * 512], oc[:]
                )

            # ---- attention: one kt step, with PE filler emitted between
            # the score matmuls and the AV matmuls ----
            def attn_step(t, qc, kt, nkt, avs, fillers):
                off = max(0, (kt - 4 * qc) * P)  # first visible q col
                ex = expp.tile([P, 2, 512], FPC, tag="ex", name="ex")
                for hl in range(2):
                    base = hl * DH
                    sc = psS.tile([P, 512], FP, tag="psS", name=f"sc{hl}")
                    nc.tensor.matmul(
                        sc[:, off:512],
                        kT_sb[t][base : base + DH, kt * P : (kt + 1) * P],
                        qT_sb[t][
                            base : base + DH,
                            qc * 512 + off : (qc + 1) * 512,
                        ],
                        start=True,
                        stop=True,
                        tile_position=(base, 0),
                    )
                    nc.scalar.activation(
                        ex[:, hl, off:512],
                        sc[:, off:512],
                        mybir.ActivationFunctionType.Exp,
                    )
                j = kt - 4 * qc
                if j >= 0:  # diagonal block gets the triangular mask
                    nc.vector.tensor_mul(
                        ex[:, :, j * P : (j + 1) * P],
                        ex[:, :, j * P : (j + 1) * P],
                        mk_sb[:].rearrange("p (o k) -> p o k", o=1).to_broadcast([P, 2, P]),
                    )
                if fillers:
                    fillers.pop(0)()
                for hl in range(2):
                    nc.tensor.matmul(
                        avs[hl][:, off:512],
                        v_sb[:, kt, 2 * t + hl, :],
                        ex[:, hl, off:512],
                        start=(kt == 0),
                        stop=(kt == nkt - 1),
                    )

            def attn_group(t, qc, fillers):
                avs = [
                    psV.tile([DH + 1, 512], FP, tag="psV", name=f"av{hl}")
                    for hl in range(2)
                ]
                nkt = 4 * qc + 4  # causal: k tiles 0..4qc+3
                for kt in range(nkt):
                    attn_step(t, qc, kt, nkt, avs, fillers)
                tail = t == 1 and qc == NQC - 1
                for hl in range(2):
                    base = hl * DH
                    # stage the denominator row to base partition 0: the
                    # custom-DVE reciprocal drops nonzero partition offsets
                    den = smp.tile([1, 512], FP, tag="den")
                    nc.vector.tensor_copy(den[:], avs[hl][DH : DH + 1, :])
                    rc = smp.tile([1, 512], FP, tag="rc")
                    nc.vector.reciprocal_approx_fast(rc[:], den[:])
                    if tail:
                        # final group gates the last Wo burst: broadcast on the
                        # PE via a K=1 matmul instead of the slower gpsimd op.
                        # Only one tensor_tensor input may live in PSUM, so
                        # stage the AV numerator to SBUF (overlaps the matmul).
                        rc16 = smp.tile([1, 512], FPC, tag="rc16")
                        nc.vector.tensor_copy(rc16[:], rc[:])
                        bcp = psS.tile([P, 512], FP, tag="psS", name="bcp")
                        nc.tensor.matmul(
                            bcp[:DH, :], ones_row[:], rc16[:], start=True, stop=True
                        )
                        avr = smp.tile([DH, 512], FP, tag="bc")
                        nc.vector.tensor_copy(avr[:], avs[hl][:DH, :])
                        nc.vector.tensor_mul(
                            aT_sb[t][qc][base : base + DH, :],
                            avr[:],
                            bcp[:DH, :],
                        )
                    else:
                        bc = smp.tile([DH, 512], FP, tag="bc")
                        nc.gpsimd.partition_broadcast(bc[:], rc[:], channels=DH)
                        nc.vector.tensor_mul(
                            aT_sb[t][qc][base : base + DH, :],
                            avs[hl][:DH, :],
                            bc[:],
                        )

            # ================= emission schedule =================
            for qc in range(NQC):
                proj(wq_sb, qT_sb, 0, qc)
            lorentz(0)
            for qc in range(NQC):
                proj(wq_sb, qT_sb, 1, qc)
            lorentz(1)
            for qc in range(NQC):
                kb = [None]
                kproj_half(0, qc, 0, kb)
                kproj_half(0, qc, 1, kb)
            for lt in range(4):
                vproj(lt)
            # preload the Exp table while the PE is still busy on projections
            dummy = smp.tile([1, 2], FPC, tag="dummy")
            nc.scalar.activation(
                dummy[:], ones_row[:, 0:2], mybir.ActivationFunctionType.Exp
            )

            # fillers for attention t0: V tail then K-proj t1
            fillers = [lambda lt=lt: vproj(lt) for lt in range(4, NKT)]
            kb0, kb1 = [None], [None]
            fillers += [
                lambda: kproj_half(1, 0, 0, kb0),
                lambda: kproj_half(1, 0, 1, kb0),
                lambda: kproj_half(1, 1, 0, kb1),
                lambda: kproj_half(1, 1, 1, kb1),
            ]
            attn_group(0, 0, fillers)
            attn_group(0, 1, fillers)
            for f in fillers:
                f()
            fillers = []
            attn_group(1, 0, fillers)
            # Wo for qc0 fills attention (t1, qc1); half the PSUM->SBUF output
            # copies go on the ACT engine, half on the DVE
            fillers = [
                lambda lt=lt, jc=jc: wo_unit(lt, jc, oc_on_act=(jc == 0))
                for lt in range(4)
                for jc in range(2)
            ]
            attn_group(1, 1, fillers)
            for f in fillers:
                f()
            for lt in range(4, NKT):
                for jc in range(2):
                    wo_unit(lt, jc, oc_on_act=(jc == 0))

            if debug:
                qTd = nc.dram_tensor("qTd", [2, P, L], FPC, kind="ExternalOutput")
                kTd = nc.dram_tensor("kTd", [2, P, L], FPC, kind="ExternalOutput")
                vd = nc.dram_tensor(
                    "vd", [P, NKT, HPC, DH + 1], FPC, kind="ExternalOutput"
                )
                aTd = nc.dram_tensor(
                    "aTd", [2, NQC, P, 512], FPC, kind="ExternalOutput"
                )
                for t in range(2):
                    nc.sync.dma_start(qTd[t], qT_sb[t][:])
                    nc.sync.dma_start(kTd[t], kT_sb[t][:])
                    for qc in range(NQC):
                        nc.sync.dma_start(aTd[t, qc], aT_sb[t][qc][:])
                nc.sync.dma_start(vd[:], v_sb[:])

    nc.compile()
    return nc


_NC = None


def _host_inputs(x, Wq, Wk, Wv, Wo, timelike_mask):
    m_full = np.asarray(timelike_mask).astype(np.float32)
    mt = np.tril(np.ones((P, P), dtype=np.float32)).T.copy()  # maskT[k,q]=1 iff k<=q
    in_maps = []
    for c in range(N_CORES):
        b, g = divmod(c, HPC)
        sl = slice(g * DPC, (g + 1) * DPC)
        m = m_full[sl]  # [256]
        nb = np.zeros((P, 2, 34), dtype=np.float32)
        sp = np.zeros((3, 2, P), dtype=np.float32)
        for t in range(2):
            m_t = m[t * P : (t + 1) * P]
            nb[0:DH, t, 0] = 1.0
            nb[DH:P, t, 1] = 1.0
            nb[0:DH, t, 32] = m_t[0:DH]
            nb[DH:P, t, 33] = m_t[DH:P]
            coef = -2.0 * ALPHA / SCALE  # -0.0625
            sp[0, t, 0:DH] = coef * m_t[0:DH]
            sp[1, t, DH:P] = coef * m_t[DH:P]
            sp[2, t, :] = 1.0 / SCALE
        in_maps.append(
            {
                "xT": np.ascontiguousarray(x[b].T).astype(NPC),
                "wqT": np.ascontiguousarray(Wq[sl, :].T).astype(NPC),
                "wkT": np.ascontiguousarray(Wk[sl, :].T).astype(NPC),
                "wvT": np.ascontiguousarray(Wv[sl, :].T).astype(NPC),
                "woT": np.ascontiguousarray(Wo[:, sl].T).astype(NPC),
                "normblk": nb.astype(NPC),
                "sprime": sp.astype(NPC),
                "maskT": mt.astype(NPC),
            }
        )
    return in_maps


def kernel(x, Wq, Wk, Wv, Wo, timelike_mask, attn_mask, _trace=False):
    global _NC
    if _NC is None:
        _NC = _build_program()
    nc = _NC

    x = np.asarray(x, dtype=np.float32)
    Wq, Wk, Wv, Wo = (np.asarray(w, dtype=np.float32) for w in (Wq, Wk, Wv, Wo))
    am = np.asarray(attn_mask, dtype=np.float32).reshape(L, L)
    causal = np.tril(np.ones((L, L), dtype=bool))
    assert np.array_equal(am, np.where(causal, 0.0, -1e9).astype(np.float32)), (
        "kernel hardcodes a causal additive mask"
    )

    in_maps = _host_inputs(x, Wq, Wk, Wv, Wo, timelike_mask)
    res = run_bass_kernel_spmd(
        nc, in_maps, core_ids=list(range(N_CORES)), trace=_trace
    )
    outp = np.stack(
        [
            sum(
                res.results[b * HPC + g]["out"].astype(np.float32)
                for g in range(HPC)
            )
            for b in range(B)
        ]
    )
    kernel.last_results = res
    return outp


# revision 17
# speedup vs baseline: 1.0852x; 1.0171x over previous
"""LorentzTransformer Trainium2 kernel.

Full inputs in, full output out. Sharding: 8 cores = 2 batches x 4 head
groups (4 heads / 256 channels each). Host pre-transposes x and the weight
shards so every on-chip matmul has its contraction dim on partitions.

Per-core pipeline (fp16 PE datapath, fp32 PSUM accumulation):
  QT/KT = W-proj of x (head channels on partitions, seq on free)
  V     = natural-layout proj, augmented with a ones column (softmax denom)
  Qeff  = Q * (0.125 - 0.0625*sf*m); sf via one M=4 PE partition-sum matmul,
  the +0.125 folded in as a third ones-row of the sprime matmul
  scoresT[k,q], head pairs row-packed on the PE -> exp on ACT -> causal via
  block skipping + one triangular 0/1 tile, N shrunk to visible columns
  AV + denom in one PSUM accumulation group; normalize straight out of PSUM
  (reciprocal + broadcast-multiply, no staging copies)
  partial out = A @ Wo_shard.T in fp16, interleaved into the second half of
  attention so the output DMA streams early; host sums the 4 head-group
  partials per batch

Scheduling: V-proj tail and K-proj(t1) are emitted as PE filler units inside
the attention t0 kt-loop (between the score matmuls and the AV matmuls), and
Wo(qc0) units fill attention t1 — the PE queue never drains while the ACT
engine computes exps, keeping the HAM clock gate warm.
"""

import numpy as np

from concourse import bacc
import concourse.tile as tile
import concourse.mybir as mybir
from concourse.bass_utils import run_bass_kernel_spmd

B, L, D, H = 2, 1024, 1024, 16
DH = D // H  # 64
ALPHA = 0.25
SCALE = float(np.sqrt(DH))  # 8.0
HPC = 4          # heads per core
DPC = HPC * DH   # 256 channels per core
N_CORES = 8
P = 128
KCH = D // P     # 8 contraction chunks
NQC = L // 512   # q chunks of 512
NKT = L // P     # k tiles of 128

FP = mybir.dt.float32
# PE compute dtype: fp16 runs the PE at full rate on the normal datapath
# (the HAM clock gate ignores fp32r matmuls and throttles to 1.2 GHz), gets
# fast-weight-load, and keeps 11 mantissa bits. PSUM accumulation is fp32.
FPC = mybir.dt.float16
NPC = np.float16


def _build_program(debug=False):
    nc = bacc.Bacc("TRN2", target_bir_lowering=False)

    xT = nc.dram_tensor("xT", [D, L], FPC, kind="ExternalInput")
    wqT = nc.dram_tensor("wqT", [D, DPC], FPC, kind="ExternalInput")
    wkT = nc.dram_tensor("wkT", [D, DPC], FPC, kind="ExternalInput")
    wvT = nc.dram_tensor("wvT", [D, DPC], FPC, kind="ExternalInput")
    woT = nc.dram_tensor("woT", [DPC, D], FPC, kind="ExternalInput")
    normblk = nc.dram_tensor("normblk", [P, 2, 34], FPC, kind="ExternalInput")
    sprime = nc.dram_tensor("sprime", [3, 2, P], FPC, kind="ExternalInput")
    maskT = nc.dram_tensor("maskT", [P, P], FPC, kind="ExternalInput")
    out = nc.dram_tensor("out", [L, D], FPC, kind="ExternalOutput")

    with tile.TileContext(nc) as tc:
        with (
            tc.tile_pool(name="persist", bufs=1) as persist,
            tc.tile_pool(name="work", bufs=2) as work,
            tc.tile_pool(name="expp", bufs=8) as expp,
            tc.tile_pool(name="sm", bufs=6) as smp,
            tc.tile_pool(name="ost", bufs=4) as ost,
            tc.tile_pool(name="psA", bufs=2, space="PSUM") as psA,
            tc.tile_pool(name="psS", bufs=3, space="PSUM") as psS,
            tc.tile_pool(name="psV", bufs=3, space="PSUM") as psV,
        ):
            # ---- persistent SBUF tiles ----
            xT_sb = persist.tile([P, KCH, L], FPC, tag="xT")
            wq_sb = persist.tile([P, KCH, DPC], FPC, tag="wq")
            wk_sb = persist.tile([P, KCH, DPC], FPC, tag="wk")
            wv_sb = persist.tile([P, KCH, DPC], FPC, tag="wv")
            wo_sb = persist.tile([P, DPC // P, D], FPC, tag="wo")
            nb_sb = persist.tile([P, 2, 34], FPC, tag="nb")
            sp_sb = persist.tile([3, 2, P], FPC, tag="sp")
            mk_sb = persist.tile([P, P], FPC, tag="mk")

            # ---- input DMA: batched, ordered so the Q-projection deps land
            # first at full HBM bandwidth; only the tiny tensors ride the
            # scalar HWDGE queue (big ones there would steal bandwidth) ----
            nc.sync.dma_start(wq_sb[:], wqT.rearrange("(o p) n -> p o n", p=P))
            xT_r = xT.rearrange("(o p) l -> p o l", p=P)
            nc.sync.dma_start(xT_sb[:, 0:4, :], xT_r[:, 0:4])
            nc.sync.dma_start(xT_sb[:, 4:8, :], xT_r[:, 4:8])
            nc.sync.dma_start(wk_sb[:], wkT.rearrange("(o p) n -> p o n", p=P))
            nc.sync.dma_start(wv_sb[:], wvT.rearrange("(o p) n -> p o n", p=P))
            nc.sync.dma_start(wo_sb[:], woT.rearrange("(o p) n -> p o n", p=P))
            nc.scalar.dma_start(nb_sb[:], normblk[:])
            nc.scalar.dma_start(sp_sb[:], sprime[:])
            nc.scalar.dma_start(mk_sb[:], maskT[:])

            qT_sb = [persist.tile([P, L], FPC, tag=f"qT{t}", name=f"qT{t}") for t in range(2)]
            kT_sb = [persist.tile([P, L], FPC, tag=f"kT{t}", name=f"kT{t}") for t in range(2)]
            # V' with ones column per (ktile, head)
            v_sb = persist.tile([P, NKT, HPC, DH + 1], FPC, tag="v")
            onecol = persist.tile([P, 1], FP, tag="onecol")
            nc.vector.memset(onecol[:], 1.0)
            nc.vector.tensor_copy(
                v_sb[:, :, :, DH : DH + 1],
                onecol.to_broadcast([P, NKT, HPC, 1]),
            )

            ones_row = persist.tile([1, DH], FPC, tag="ones_row")
            nc.vector.memset(ones_row[:], 1.0)

            aT_sb = [
                [
                    persist.tile([P, 512], FPC, tag=f"aT{t}_{qc}", name=f"aT{t}_{qc}")
                    for qc in range(NQC)
                ]
                for t in range(2)
            ]

            # ---- projections ----
            def proj(w_sb, dst, t, qc):
                ps = psA.tile([P, 512], FP, tag="psA", name="proj")
                for k in range(KCH):
                    nc.tensor.matmul(
                        ps[:],
                        w_sb[:, k, t * P : (t + 1) * P],
                        xT_sb[:, k, qc * 512 : (qc + 1) * 512],
                        start=(k == 0),
                        stop=(k == KCH - 1),
                    )
                nc.vector.tensor_copy(dst[t][:, qc * 512 : (qc + 1) * 512], ps[:])

            # lorentz: QeffT = QT * (0.125 - 0.0625*sf*m), sf = |Q|/|Qt| per
            # (head, q). Split into pieces so PE work can be emitted between
            # the serial DVE/ACT chain segments.
            sq_t = [None, None]
            sf_t = [None, None]

            def lor_sq(t):
                sq_t[t] = work.tile([P, L], FPC, tag=f"sq{t}", name=f"sq{t}")
                nc.scalar.square(sq_t[t][:], qT_sb[t][:])
                sf_t[t] = work.tile([3, L], FPC, tag=f"sf{t}", name=f"sf{t}")
                # base partition of an access must be 32-aligned: set the
                # whole 3-row tile to 1.0; rows 0:2 get overwritten below
                nc.vector.memset(sf_t[t][:], 1.0)

            def lor_nrm(t, qc):
                # one M=34 matmul: rows 0,1 = |Q|^2 per head, rows 32,33 =
                # |Qt|^2 per head (operand bases must be 32-aligned)
                nrm = psS.tile([P, 512], FP, tag="psS", name="nrm")
                nc.tensor.matmul(
                    nrm[:34, :],
                    nb_sb[:, t, :],
                    sq_t[t][:, qc * 512 : (qc + 1) * 512],
                    start=True,
                    stop=True,
                )
                # custom-DVE ops silently drop a nonzero partition offset:
                # stage rows 32:34 to a base-0 SBUF tile first
                nq2 = smp.tile([2, 512], FP, tag="nq2")
                nc.vector.tensor_copy(nq2[:], nrm[32:34, :])
                brcp = smp.tile([2, 512], FP, tag="brcp")
                nc.vector.reciprocal_approx_fast(brcp[:], nq2[:])
                rat = smp.tile([2, 512], FP, tag="rat")
                nc.vector.tensor_mul(rat[:], nrm[0:2, :], brcp[:])
                nc.scalar.activation(
                    sf_t[t][0:2, qc * 512 : (qc + 1) * 512],
                    rat[:],
                    mybir.ActivationFunctionType.Sqrt,
                )

            def lor_gps(t, qc):
                gps = psS.tile([P, 512], FP, tag="psS", name="gps")
                nc.tensor.matmul(
                    gps[:],
                    sp_sb[:, t, :],
                    sf_t[t][:, qc * 512 : (qc + 1) * 512],
                    start=True,
                    stop=True,
                )
                nc.vector.tensor_mul(
                    qT_sb[t][:, qc * 512 : (qc + 1) * 512],
                    qT_sb[t][:, qc * 512 : (qc + 1) * 512],
                    gps[:],
                )

            # ---- V natural layout: out[l, dv], packed into V' ----
            def vproj(lt):
                ps = psA.tile([P, 512], FP, tag="psA", name="vproj")
                for k in range(KCH):
                    nc.tensor.matmul(
                        ps[:, :DPC],
                        xT_sb[:, k, lt * P : (lt + 1) * P],
                        wv_sb[:, k, :],
                        start=(k == 0),
                        stop=(k == KCH - 1),
                    )
                nc.vector.tensor_copy(
                    v_sb[:, lt, :, :DH],
                    ps[:, :DPC].rearrange("p (h d) -> p h d", h=HPC),
                )

            def kproj_half(t, qc, half, ps_box):
                # half 0: open the psA group, ks 0..3; half 1: ks 4..7 + copy
                if half == 0:
                    ps_box[0] = psA.tile([P, 512], FP, tag="psA", name="kproj")
                ps = ps_box[0]
                for k in range(half * 4, half * 4 + 4):
                    nc.tensor.matmul(
                        ps[:],
                        wk_sb[:, k, t * P : (t + 1) * P],
                        xT_sb[:, k, qc * 512 : (qc + 1) * 512],
                        start=(k == 0),
                        stop=(k == KCH - 1),
                    )
                if half == 1:
                    nc.vector.tensor_copy(
                        kT_sb[t][:, qc * 512 : (qc + 1) * 512], ps[:]
                    )

            # ---- Wo partial for one (lt, jc) output tile ----
            def wo_unit(lt, jc, oc_on_act):
                qc = lt // 4
                ps = psA.tile([P, 512], FP, tag="psA", name="wops")
                for t2 in range(2):
                    nc.tensor.matmul(
                        ps[:],
                        aT_sb[t2][qc][:, (lt % 4) * P : (lt % 4 + 1) * P],
                        wo_sb[:, t2, jc * 512 : (jc + 1) * 512],
                        start=(t2 == 0),
                        stop=(t2 == 1),
                    )
                oc = ost.tile([P, 512], FPC, tag="oc")
                if oc_on_act:
                    nc.scalar.activation(
                        oc[:], ps[:], mybir.ActivationFunctionType.Copy
                    )
                else:
                    nc.vector.tensor_copy(oc[:], ps[:])
                nc.sync.dma_start(
                    out[lt * P : (lt + 1) * P, jc * 512 : (jc + 1) * 512], oc[:]
                )

            # ---- attention: one kt step, with PE filler emitted between
            # the score matmuls and the AV matmuls ----
            def attn_step(t, qc, kt, nkt, avs, fillers):
                off = max(0, (kt - 4 * qc) * P)  # first visible q col
                ex = expp.tile([P, 2, 512], FPC, tag="ex", name="ex")
                for hl in range(2):
                    base = hl * DH
                    sc = psS.tile([P, 512], FP, tag="psS", name=f"sc{hl}")
                    nc.tensor.matmul(
                        sc[:, off:512],
                        kT_sb[t][base : base + DH, kt * P : (kt + 1) * P],
                        qT_sb[t][
                            base : base + DH,
                            qc * 512 + off : (qc + 1) * 512,
                        ],
                        start=True,
                        stop=True,
                        tile_position=(base, 0),
                    )
                    nc.scalar.activation(
                        ex[:, hl, off:512],
                        sc[:, off:512],
                        mybir.ActivationFunctionType.Exp,
                    )
                j = kt - 4 * qc
                if j >= 0:  # diagonal block gets the triangular mask
                    nc.vector.tensor_mul(
                        ex[:, :, j * P : (j + 1) * P],
                        ex[:, :, j * P : (j + 1) * P],
                        mk_sb[:].rearrange("p (o k) -> p o k", o=1).to_broadcast([P, 2, P]),
                    )
                if fillers:
                    fillers.pop(0)()
                for hl in range(2):
                    nc.tensor.matmul(
                        avs[hl][:, off:512],
                        v_sb[:, kt, 2 * t + hl, :],
                        ex[:, hl, off:512],
                        start=(kt == 0),
                        stop=(kt == nkt - 1),
                    )

            def attn_group(t, qc, fillers):
                avs = [
                    psV.tile([DH + 1, 512], FP, tag="psV", name=f"av{hl}")
                    for hl in range(2)
                ]
                nkt = 4 * qc + 4  # causal: k tiles 0..4qc+3
                for kt in range(nkt):
                    attn_step(t, qc, kt, nkt, avs, fillers)
                # leftover fillers keep the PE busy during the DVE-side
                # normalization chain below
                while fillers:
                    fillers.pop(0)()
                tail = t == 1 and qc == NQC - 1
                for hl in range(2):
                    base = hl * DH
                    # stage the denominator row to base partition 0: the
                    # custom-DVE reciprocal drops nonzero partition offsets
                    den = smp.tile([1, 512], FP, tag="den")
                    nc.vector.tensor_copy(den[:], avs[hl][DH : DH + 1, :])
                    rc = smp.tile([1, 512], FP, tag="rc")
                    nc.vector.reciprocal_approx_fast(rc[:], den[:])
                    if tail:
                        # final group gates the last Wo burst: broadcast on the
                        # PE via a K=1 matmul instead of the slower gpsimd op.
                        # Only one tensor_tensor input may live in PSUM, so
                        # stage the AV numerator to SBUF (overlaps the matmul).
                        rc16 = smp.tile([1, 512], FPC, tag="rc16")
                        nc.vector.tensor_copy(rc16[:], rc[:])
                        bcp = psS.tile([P, 512], FP, tag="psS", name="bcp")
                        nc.tensor.matmul(
                            bcp[:DH, :], ones_row[:], rc16[:], start=True, stop=True
                        )
                        avr = smp.tile([DH, 512], FP, tag="bc")
                        nc.vector.tensor_copy(avr[:], avs[hl][:DH, :])
                        nc.vector.tensor_mul(
                            aT_sb[t][qc][base : base + DH, :],
                            avr[:],
                            bcp[:DH, :],
                        )
                    else:
                        bc = smp.tile([DH, 512], FP, tag="bc")
                        nc.gpsimd.partition_broadcast(bc[:], rc[:], channels=DH)
                        nc.vector.tensor_mul(
                            aT_sb[t][qc][base : base + DH, :],
                            avs[hl][:DH, :],
                            bc[:],
                        )

            # ================= emission schedule =================
            # Q projections for both t-tiles back to back (PE dense), then the
            # lorentz chains with K/V projections emitted as PE cover for the
            # serial DVE/ACT segments.
            for t in range(2):
                for qc in range(NQC):
                    proj(wq_sb, qT_sb, t, qc)
            lor_sq(0)
            lor_sq(1)
            for t in range(2):
                for qc in range(NQC):
                    lor_nrm(t, qc)
            kb00 = [None]
            kproj_half(0, 0, 0, kb00)  # PE cover while the sqrts run
            kproj_half(0, 0, 1, kb00)
            for t in range(2):
                for qc in range(NQC):
                    lor_gps(t, qc)
            vproj(0)
            # preload the Exp table (single-entry table cache: all Square/Sqrt
            # uses are behind us) while the PE chews on attention fillers
            dummy = smp.tile([1, 2], FPC, tag="dummy")
            nc.scalar.activation(
                dummy[:], ones_row[:, 0:2], mybir.ActivationFunctionType.Exp
            )

            # attention order (0,0) -> (1,0) -> (0,1) -> (1,1): every group
            # gets PE filler units, and Wo(qc0) is ready halfway through
            kb10, kb01, kb11 = [None], [None], [None]
            attn_group(0, 0, [
                lambda: vproj(1),
                lambda: vproj(2),
                lambda: vproj(3),
                lambda: kproj_half(1, 0, 0, kb10),
                lambda: kproj_half(1, 0, 1, kb10),
            ])
            attn_group(1, 0, [
                lambda: kproj_half(0, 1, 0, kb01),
                lambda: kproj_half(0, 1, 1, kb01),
                lambda: vproj(4),
                lambda: vproj(5),
            ])
            attn_group(0, 1, [
                lambda: vproj(6),
                lambda: vproj(7),
                lambda: kproj_half(1, 1, 0, kb11),
                lambda: kproj_half(1, 1, 1, kb11),
                lambda: wo_unit(0, 0, False),
                lambda: wo_unit(0, 1, True),
                lambda: wo_unit(1, 0, False),
                lambda: wo_unit(1, 1, True),
            ])
            attn_group(1, 1, [
                lambda: wo_unit(2, 0, False),
                lambda: wo_unit(2, 1, True),
                lambda: wo_unit(3, 0, False),
                lambda: wo_unit(3, 1, True),
            ])
            for lt in range(4, NKT):
                for jc in range(2):
                    wo_unit(lt, jc, oc_on_act=(jc == 0))

            if debug:
                qTd = nc.dram_tensor("qTd", [2, P, L], FPC, kind="ExternalOutput")
                kTd = nc.dram_tensor("kTd", [2, P, L], FPC, kind="ExternalOutput")
                vd = nc.dram_tensor(
                    "vd", [P, NKT, HPC, DH + 1], FPC, kind="ExternalOutput"
                )
                aTd = nc.dram_tensor(
                    "aTd", [2, NQC, P, 512], FPC, kind="ExternalOutput"
                )
                for t in range(2):
                    nc.sync.dma_start(qTd[t], qT_sb[t][:])
                    nc.sync.dma_start(kTd[t], kT_sb[t][:])
                    for qc in range(NQC):
                        nc.sync.dma_start(aTd[t, qc], aT_sb[t][qc][:])
                nc.sync.dma_start(vd[:], v_sb[:])

    nc.compile()
    return nc


_NC = None


def _host_inputs(x, Wq, Wk, Wv, Wo, timelike_mask):
    m_full = np.asarray(timelike_mask).astype(np.float32)
    mt = np.tril(np.ones((P, P), dtype=np.float32)).T.copy()  # maskT[k,q]=1 iff k<=q
    in_maps = []
    for c in range(N_CORES):
        b, g = divmod(c, HPC)
        sl = slice(g * DPC, (g + 1) * DPC)
        m = m_full[sl]  # [256]
        nb = np.zeros((P, 2, 34), dtype=np.float32)
        sp = np.zeros((3, 2, P), dtype=np.float32)
        for t in range(2):
            m_t = m[t * P : (t + 1) * P]
            nb[0:DH, t, 0] = 1.0
            nb[DH:P, t, 1] = 1.0
            nb[0:DH, t, 32] = m_t[0:DH]
            nb[DH:P, t, 33] = m_t[DH:P]
            coef = -2.0 * ALPHA / SCALE  # -0.0625
            sp[0, t, 0:DH] = coef * m_t[0:DH]
            sp[1, t, DH:P] = coef * m_t[DH:P]
            sp[2, t, :] = 1.0 / SCALE
        in_maps.append(
            {
                "xT": np.ascontiguousarray(x[b].T).astype(NPC),
                "wqT": np.ascontiguousarray(Wq[sl, :].T).astype(NPC),
                "wkT": np.ascontiguousarray(Wk[sl, :].T).astype(NPC),
                "wvT": np.ascontiguousarray(Wv[sl, :].T).astype(NPC),
                "woT": np.ascontiguousarray(Wo[:, sl].T).astype(NPC),
                "normblk": nb.astype(NPC),
                "sprime": sp.astype(NPC),
                "maskT": mt.astype(NPC),
            }
        )
    return in_maps


def kernel(x, Wq, Wk, Wv, Wo, timelike_mask, attn_mask, _trace=False):
    global _NC
    if _NC is None:
        _NC = _build_program()
    nc = _NC

    x = np.asarray(x, dtype=np.float32)
    Wq, Wk, Wv, Wo = (np.asarray(w, dtype=np.float32) for w in (Wq, Wk, Wv, Wo))
    am = np.asarray(attn_mask, dtype=np.float32).reshape(L, L)
    causal = np.tril(np.ones((L, L), dtype=bool))
    assert np.array_equal(am, np.where(causal, 0.0, -1e9).astype(np.float32)), (
        "kernel hardcodes a causal additive mask"
    )

    in_maps = _host_inputs(x, Wq, Wk, Wv, Wo, timelike_mask)
    res = run_bass_kernel_spmd(
        nc, in_maps, core_ids=list(range(N_CORES)), trace=_trace
    )
    outp = np.stack(
        [
            sum(
                res.results[b * HPC + g]["out"].astype(np.float32)
                for g in range(HPC)
            )
            for b in range(B)
        ]
    )
    kernel.last_results = res
    return outp


# revision 24
# speedup vs baseline: 1.1323x; 1.0434x over previous
"""LorentzTransformer Trainium2 kernel.

Full inputs in, full output out. Sharding: 8 cores = 2 batches x 4 head
groups (4 heads / 256 channels each). Host pre-transposes x and the weight
shards so every on-chip matmul has its contraction dim on partitions.

Per-core pipeline (fp16 PE datapath, fp32 PSUM accumulation):
  QT/KT = W-proj of x (head channels on partitions, seq on free)
  V     = natural-layout proj, augmented with a ones column (softmax denom)
  Qeff  = Q * (0.125 - 0.0625*sf*m); sf via one M=4 PE partition-sum matmul,
  the +0.125 folded in as a third ones-row of the sprime matmul
  scoresT[k,q], head pairs row-packed on the PE -> exp on ACT -> causal via
  block skipping + one triangular 0/1 tile, N shrunk to visible columns
  AV + denom in one PSUM accumulation group; normalize straight out of PSUM
  (reciprocal + broadcast-multiply, no staging copies)
  partial out = A @ Wo_shard.T in fp16, interleaved into the second half of
  attention so the output DMA streams early; host sums the 4 head-group
  partials per batch

Scheduling: V-proj tail and K-proj(t1) are emitted as PE filler units inside
the attention t0 kt-loop (between the score matmuls and the AV matmuls), and
Wo(qc0) units fill attention t1 — the PE queue never drains while the ACT
engine computes exps, keeping the HAM clock gate warm.
"""

import numpy as np

from concourse import bacc
import concourse.tile as tile
import concourse.mybir as mybir
from concourse.bass_utils import run_bass_kernel_spmd

B, L, D, H = 2, 1024, 1024, 16
DH = D // H  # 64
ALPHA = 0.25
SCALE = float(np.sqrt(DH))  # 8.0
HPC = 4          # heads per core
DPC = HPC * DH   # 256 channels per core
N_CORES = 8
P = 128
KCH = D // P     # 8 contraction chunks
NQC = L // 512   # q chunks of 512
NKT = L // P     # k tiles of 128

FP = mybir.dt.float32
# PE compute dtype: fp16 runs the PE at full rate on the normal datapath
# (the HAM clock gate ignores fp32r matmuls and throttles to 1.2 GHz), gets
# fast-weight-load, and keeps 11 mantissa bits. PSUM accumulation is fp32.
FPC = mybir.dt.float16
NPC = np.float16


def _build_program(debug=False):
    nc = bacc.Bacc("TRN2", target_bir_lowering=False)

    xT = nc.dram_tensor("xT", [D, L], FPC, kind="ExternalInput")
    wqT = nc.dram_tensor("wqT", [D, DPC], FPC, kind="ExternalInput")
    wkT = nc.dram_tensor("wkT", [D, DPC], FPC, kind="ExternalInput")
    wvT = nc.dram_tensor("wvT", [D, DPC], FPC, kind="ExternalInput")
    woT = nc.dram_tensor("woT", [DPC, D], FPC, kind="ExternalInput")
    normblk = nc.dram_tensor("normblk", [P, 2, 34], FPC, kind="ExternalInput")
    sprime = nc.dram_tensor("sprime", [3, 2, P], FPC, kind="ExternalInput")
    maskT = nc.dram_tensor("maskT", [P, P], FPC, kind="ExternalInput")
    out = nc.dram_tensor("out", [L, D], FPC, kind="ExternalOutput")

    with tile.TileContext(nc) as tc:
        with (
            tc.tile_pool(name="persist", bufs=1) as persist,
            tc.tile_pool(name="work", bufs=2) as work,
            tc.tile_pool(name="expp", bufs=8) as expp,
            tc.tile_pool(name="sm", bufs=6) as smp,
            tc.tile_pool(name="ost", bufs=4) as ost,
            tc.tile_pool(name="psA", bufs=2, space="PSUM") as psA,
            tc.tile_pool(name="psS", bufs=3, space="PSUM") as psS,
            tc.tile_pool(name="psV", bufs=3, space="PSUM") as psV,
        ):
            # ---- persistent SBUF tiles ----
            xT_sb = persist.tile([P, KCH, L], FPC, tag="xT")
            wq_sb = persist.tile([P, KCH, DPC], FPC, tag="wq")
            wk_sb = persist.tile([P, KCH, DPC], FPC, tag="wk")
            wv_sb = persist.tile([P, KCH, DPC], FPC, tag="wv")
            wo_sb = persist.tile([P, DPC // P, D], FPC, tag="wo")
            nb_sb = persist.tile([P, 2, 34], FPC, tag="nb")
            sp_sb = persist.tile([3, 2, P], FPC, tag="sp")
            mk_sb = persist.tile([P, P], FPC, tag="mk")

            # ---- input DMA: batched, ordered so the Q-projection deps land
            # first at full HBM bandwidth; only the tiny tensors ride the
            # scalar HWDGE queue (big ones there would steal bandwidth) ----
            # each HWDGE queue sustains only ~270 GB/s: split the load stream
            # across both, Q-projection dependencies first on each
            nc.sync.dma_start(wq_sb[:], wqT.rearrange("(o p) n -> p o n", p=P))
            xT_r = xT.rearrange("(o p) l -> p o l", p=P)
            nc.sync.dma_start(xT_sb[:, 0:2, :], xT_r[:, 0:2])
            nc.sync.dma_start(xT_sb[:, 2:4, :], xT_r[:, 2:4])
            nc.sync.dma_start(wk_sb[:], wkT.rearrange("(o p) n -> p o n", p=P))
            nc.scalar.dma_start(nb_sb[:], normblk[:])
            nc.scalar.dma_start(sp_sb[:], sprime[:])
            nc.scalar.dma_start(mk_sb[:], maskT[:])
            nc.scalar.dma_start(xT_sb[:, 4:6, :], xT_r[:, 4:6])
            nc.scalar.dma_start(xT_sb[:, 6:8, :], xT_r[:, 6:8])
            nc.scalar.dma_start(wv_sb[:], wvT.rearrange("(o p) n -> p o n", p=P))
            nc.scalar.dma_start(wo_sb[:], woT.rearrange("(o p) n -> p o n", p=P))

            qT_sb = [persist.tile([P, L], FPC, tag=f"qT{t}", name=f"qT{t}") for t in range(2)]
            kT_sb = [persist.tile([P, L], FPC, tag=f"kT{t}", name=f"kT{t}") for t in range(2)]
            # V' with ones column per (ktile, head)
            v_sb = persist.tile([P, NKT, HPC, DH + 1], FPC, tag="v")
            onecol = persist.tile([P, 1], FP, tag="onecol")
            nc.vector.memset(onecol[:], 1.0)
            nc.vector.tensor_copy(
                v_sb[:, :, :, DH : DH + 1],
                onecol.to_broadcast([P, NKT, HPC, 1]),
            )

            ones_row = persist.tile([1, DH], FPC, tag="ones_row")
            nc.vector.memset(ones_row[:], 1.0)

            # sf tiles: rows 0,1 = per-head |Q|/|Qt| (sqrt writes them), row 2
            # stays 1.0 so the sprime matmul folds in the +1/SCALE constant.
            # Allocated + memset early while the DVE is otherwise idle (a
            # base-partition-2 single-row memset would be illegal).
            sf_t = [
                persist.tile([3, L], FPC, tag=f"sf{t}", name=f"sf{t}")
                for t in range(2)
            ]
            for t in range(2):
                nc.vector.memset(sf_t[t][:], 1.0)

            aT_sb = [
                [
                    persist.tile([P, 512], FPC, tag=f"aT{t}_{qc}", name=f"aT{t}_{qc}")
                    for qc in range(NQC)
                ]
                for t in range(2)
            ]

            # ---- projections ----
            def proj(w_sb, dst, t, qc):
                ps = psA.tile([P, 512], FP, tag="psA", name="proj")
                for k in range(KCH):
                    nc.tensor.matmul(
                        ps[:],
                        w_sb[:, k, t * P : (t + 1) * P],
                        xT_sb[:, k, qc * 512 : (qc + 1) * 512],
                        start=(k == 0),
                        stop=(k == KCH - 1),
                    )
                nc.vector.tensor_copy(dst[t][:, qc * 512 : (qc + 1) * 512], ps[:])

            # lorentz: QeffT = QT * (0.125 - 0.0625*sf*m), sf = |Q|/|Qt| per
            # (head, q). Split into pieces so PE work can be emitted between
            # the serial DVE/ACT chain segments.
            sq_t = [None, None]

            def lor_sq(t):
                sq_t[t] = work.tile([P, L], FPC, tag=f"sq{t}", name=f"sq{t}")
                nc.scalar.square(sq_t[t][:], qT_sb[t][:])

            def lor_nrm(t, qc):
                # one M=34 matmul: rows 0,1 = |Qt|^2 per head (base 0 so the
                # custom-DVE reciprocal can read it directly), rows 32,33 =
                # |Q|^2 per head (regular DVE ops handle the offset fine)
                nrm = psS.tile([P, 512], FP, tag="psS", name="nrm")
                nc.tensor.matmul(
                    nrm[:34, :],
                    nb_sb[:, t, :],
                    sq_t[t][:, qc * 512 : (qc + 1) * 512],
                    start=True,
                    stop=True,
                )
                brcp = smp.tile([2, 512], FP, tag="brcp")
                nc.vector.reciprocal_approx_fast(brcp[:], nrm[0:2, :])
                rat = smp.tile([2, 512], FP, tag="rat")
                nc.vector.tensor_mul(rat[:], nrm[32:34, :], brcp[:])
                nc.scalar.activation(
                    sf_t[t][0:2, qc * 512 : (qc + 1) * 512],
                    rat[:],
                    mybir.ActivationFunctionType.Sqrt,
                )

            def lor_gps(t, qc):
                gps = psS.tile([P, 512], FP, tag="psS", name="gps")
                nc.tensor.matmul(
                    gps[:],
                    sp_sb[:, t, :],
                    sf_t[t][:, qc * 512 : (qc + 1) * 512],
                    start=True,
                    stop=True,
                )
                nc.vector.tensor_mul(
                    qT_sb[t][:, qc * 512 : (qc + 1) * 512],
                    qT_sb[t][:, qc * 512 : (qc + 1) * 512],
                    gps[:],
                )

            # ---- V natural layout: out[l, dv], packed into V' ----
            def vproj(lt):
                ps = psA.tile([P, 512], FP, tag="psA", name="vproj")
                for k in range(KCH):
                    nc.tensor.matmul(
                        ps[:, :DPC],
                        xT_sb[:, k, lt * P : (lt + 1) * P],
                        wv_sb[:, k, :],
                        start=(k == 0),
                        stop=(k == KCH - 1),
                    )
                nc.vector.tensor_copy(
                    v_sb[:, lt, :, :DH],
                    ps[:, :DPC].rearrange("p (h d) -> p h d", h=HPC),
                )

            def kproj_half(t, qc, half, ps_box):
                # half 0: open the psA group, ks 0..3; half 1: ks 4..7 + copy
                if half == 0:
                    ps_box[0] = psA.tile([P, 512], FP, tag="psA", name="kproj")
                ps = ps_box[0]
                for k in range(half * 4, half * 4 + 4):
                    nc.tensor.matmul(
                        ps[:],
                        wk_sb[:, k, t * P : (t + 1) * P],
                        xT_sb[:, k, qc * 512 : (qc + 1) * 512],
                        start=(k == 0),
                        stop=(k == KCH - 1),
                    )
                if half == 1:
                    nc.vector.tensor_copy(
                        kT_sb[t][:, qc * 512 : (qc + 1) * 512], ps[:]
                    )

            # ---- Wo partial for one (lt, jc) output tile ----
            def wo_unit(lt, jc, oc_on_act):
                qc = lt // 4
                ps = psA.tile([P, 512], FP, tag="psA", name="wops")
                for t2 in range(2):
                    nc.tensor.matmul(
                        ps[:],
                        aT_sb[t2][qc][:, (lt % 4) * P : (lt % 4 + 1) * P],
                        wo_sb[:, t2, jc * 512 : (jc + 1) * 512],
                        start=(t2 == 0),
                        stop=(t2 == 1),
                    )
                oc = ost.tile([P, 512], FPC, tag="oc")
                if oc_on_act:
                    nc.scalar.activation(
                        oc[:], ps[:], mybir.ActivationFunctionType.Copy
                    )
                else:
                    nc.vector.tensor_copy(oc[:], ps[:])
                nc.sync.dma_start(
                    out[lt * P : (lt + 1) * P, jc * 512 : (jc + 1) * 512], oc[:]
                )

            # ---- attention: one kt step, with PE filler emitted between
            # the score matmuls and the AV matmuls ----
            def attn_step(t, qc, kt, nkt, avs, fillers):
                off = max(0, (kt - 4 * qc) * P)  # first visible q col
                ex = expp.tile([P, 2, 512], FPC, tag="ex", name="ex")
                for hl in range(2):
                    base = hl * DH
                    sc = psS.tile([P, 512], FP, tag="psS", name=f"sc{hl}")
                    nc.tensor.matmul(
                        sc[:, off:512],
                        kT_sb[t][base : base + DH, kt * P : (kt + 1) * P],
                        qT_sb[t][
                            base : base + DH,
                            qc * 512 + off : (qc + 1) * 512,
                        ],
                        start=True,
                        stop=True,
                        tile_position=(base, 0),
                    )
                    nc.scalar.activation(
                        ex[:, hl, off:512],
                        sc[:, off:512],
                        mybir.ActivationFunctionType.Exp,
                    )
                j = kt - 4 * qc
                if j >= 0:  # diagonal block gets the triangular mask
                    nc.vector.tensor_mul(
                        ex[:, :, j * P : (j + 1) * P],
                        ex[:, :, j * P : (j + 1) * P],
                        mk_sb[:].rearrange("p (o k) -> p o k", o=1).to_broadcast([P, 2, P]),
                    )
                if fillers:
                    fillers.pop(0)()
                for hl in range(2):
                    nc.tensor.matmul(
                        avs[hl][:, off:512],
                        v_sb[:, kt, 2 * t + hl, :],
                        ex[:, hl, off:512],
                        start=(kt == 0),
                        stop=(kt == nkt - 1),
                    )

            def attn_group(t, qc, fillers):
                avs = [
                    psV.tile([DH + 1, 512], FP, tag="psV", name=f"av{hl}")
                    for hl in range(2)
                ]
                nkt = 4 * qc + 4  # causal: k tiles 0..4qc+3
                for kt in range(nkt):
                    attn_step(t, qc, kt, nkt, avs, fillers)
                # leftover fillers keep the PE busy during the DVE-side
                # normalization chain below
                while fillers:
                    fillers.pop(0)()
                tail = t == 1 and qc == NQC - 1
                for hl in range(2):
                    base = hl * DH
                    # stage the denominator row to base partition 0: the
                    # custom-DVE reciprocal drops nonzero partition offsets.
                    # For the tail group the copies ride the (now idle) ACT
                    # engine so the serial-DVE chain shrinks to recip+mul.
                    den = smp.tile([1, 512], FP, tag="den")
                    if tail:
                        nc.scalar.activation(
                            den[:],
                            avs[hl][DH : DH + 1, :],
                            mybir.ActivationFunctionType.Copy,
                        )
                    else:
                        nc.vector.tensor_copy(den[:], avs[hl][DH : DH + 1, :])
                    rc = smp.tile([1, 512], FP, tag="rc")
                    nc.vector.reciprocal_approx_fast(rc[:], den[:])
                    if tail:
                        # final group gates the last Wo burst: broadcast on the
                        # PE via a K=1 matmul instead of the slower gpsimd op.
                        # Only one tensor_tensor input may live in PSUM, so
                        # stage the AV numerator to SBUF (overlaps the matmul).
                        rc16 = smp.tile([1, 512], FPC, tag="rc16")
                        nc.scalar.activation(
                            rc16[:], rc[:], mybir.ActivationFunctionType.Copy
                        )
                        bcp = psS.tile([P, 512], FP, tag="psS", name="bcp")
                        nc.tensor.matmul(
                            bcp[:DH, :], ones_row[:], rc16[:], start=True, stop=True
                        )
                        avr = smp.tile([DH, 512], FP, tag="bc")
                        nc.scalar.activation(
                            avr[:],
                            avs[hl][:DH, :],
                            mybir.ActivationFunctionType.Copy,
                        )
                        nc.vector.tensor_mul(
                            aT_sb[t][qc][base : base + DH, :],
                            avr[:],
                            bcp[:DH, :],
                        )
                    else:
                        bc = smp.tile([DH, 512], FP, tag="bc")
                        nc.gpsimd.partition_broadcast(bc[:], rc[:], channels=DH)
                        nc.vector.tensor_mul(
                            aT_sb[t][qc][base : base + DH, :],
                            avs[hl][:DH, :],
                            bc[:],
                        )

            # ================= emission schedule =================
            # Q projections for both t-tiles back to back (PE dense), then the
            # lorentz chains with K/V projections emitted as PE cover for the
            # serial DVE/ACT segments.
            for t in range(2):
                for qc in range(NQC):
                    proj(wq_sb, qT_sb, t, qc)
            lor_sq(0)
            lor_sq(1)
            for t in range(2):
                for qc in range(NQC):
                    lor_nrm(t, qc)
            # PE cover for the serial recip/mul/sqrt chains above
            kb00, kb01 = [None], [None]
            kproj_half(0, 0, 0, kb00)
            kproj_half(0, 0, 1, kb00)
            kproj_half(0, 1, 0, kb01)
            kproj_half(0, 1, 1, kb01)
            vproj(0)
            for t in range(2):
                for qc in range(NQC):
                    lor_gps(t, qc)
            vproj(1)
            # preload the Exp table (single-entry table cache: all Square/Sqrt
            # uses are behind us) while the PE chews on attention fillers
            dummy = smp.tile([1, 2], FPC, tag="dummy")
            nc.scalar.activation(
                dummy[:], ones_row[:, 0:2], mybir.ActivationFunctionType.Exp
            )

            # attention order (0,0) -> (1,0) -> (0,1) -> (1,1): every group
            # gets PE filler units, and Wo(qc0) is ready halfway through
            kb10, kb11 = [None], [None]
            attn_group(0, 0, [
                lambda: vproj(2),
                lambda: vproj(3),
                lambda: kproj_half(1, 0, 0, kb10),
                lambda: kproj_half(1, 0, 1, kb10),
            ])
            attn_group(1, 0, [
                lambda: vproj(4),
                lambda: vproj(5),
                lambda: vproj(6),
            ])
            attn_group(0, 1, [
                lambda: vproj(7),
                lambda: kproj_half(1, 1, 0, kb11),
                lambda: kproj_half(1, 1, 1, kb11),
                lambda: wo_unit(0, 0, False),
                lambda: wo_unit(0, 1, True),
                lambda: wo_unit(1, 0, False),
                lambda: wo_unit(1, 1, True),
            ])
            attn_group(1, 1, [
                lambda: wo_unit(2, 0, False),
                lambda: wo_unit(2, 1, True),
                lambda: wo_unit(3, 0, False),
                lambda: wo_unit(3, 1, True),
            ])
            # final Wo burst: open the t2=0 halves of four accumulation groups
            # first (2 psA + 2 psS banks) so the PE runs them during the tail
            # normalization chain; the t2=1 halves land once aT(1,1) is ready
            lts = [(lt, jc) for lt in range(4, NKT) for jc in range(2)]
            open_ps = []
            for u, (lt, jc) in enumerate(lts[:4]):
                pool = psA if u % 2 == 0 else psS
                tag = "psA" if u % 2 == 0 else "psS"
                ps = pool.tile([P, 512], FP, tag=tag, name=f"wof{u}")
                nc.tensor.matmul(
                    ps[:],
                    aT_sb[0][1][:, (lt % 4) * P : (lt % 4 + 1) * P],
                    wo_sb[:, 0, jc * 512 : (jc + 1) * 512],
                    start=True,
                    stop=False,
                )
                open_ps.append(ps)
            for u, (lt, jc) in enumerate(lts[:4]):
                nc.tensor.matmul(
                    open_ps[u],
                    aT_sb[1][1][:, (lt % 4) * P : (lt % 4 + 1) * P],
                    wo_sb[:, 1, jc * 512 : (jc + 1) * 512],
                    start=False,
                    stop=True,
                )
                oc = ost.tile([P, 512], FPC, tag="oc")
                if u % 2 == 0:
                    nc.vector.tensor_copy(oc[:], open_ps[u])
                else:
                    nc.scalar.activation(
                        oc[:], open_ps[u], mybir.ActivationFunctionType.Copy
                    )
                nc.sync.dma_start(
                    out[lt * P : (lt + 1) * P, jc * 512 : (jc + 1) * 512], oc[:]
                )
            for lt, jc in lts[4:]:
                wo_unit(lt, jc, oc_on_act=(jc == 0))

            if debug:
                qTd = nc.dram_tensor("qTd", [2, P, L], FPC, kind="ExternalOutput")
                kTd = nc.dram_tensor("kTd", [2, P, L], FPC, kind="ExternalOutput")
                vd = nc.dram_tensor(
                    "vd", [P, NKT, HPC, DH + 1], FPC, kind="ExternalOutput"
                )
                aTd = nc.dram_tensor(
                    "aTd", [2, NQC, P, 512], FPC, kind="ExternalOutput"
                )
                for t in range(2):
                    nc.sync.dma_start(qTd[t], qT_sb[t][:])
                    nc.sync.dma_start(kTd[t], kT_sb[t][:])
                    for qc in range(NQC):
                        nc.sync.dma_start(aTd[t, qc], aT_sb[t][qc][:])
                nc.sync.dma_start(vd[:], v_sb[:])

    nc.compile()
    return nc


_NC = None


def _host_inputs(x, Wq, Wk, Wv, Wo, timelike_mask):
    m_full = np.asarray(timelike_mask).astype(np.float32)
    mt = np.tril(np.ones((P, P), dtype=np.float32)).T.copy()  # maskT[k,q]=1 iff k<=q
    in_maps = []
    for c in range(N_CORES):
        b, g = divmod(c, HPC)
        sl = slice(g * DPC, (g + 1) * DPC)
        m = m_full[sl]  # [256]
        nb = np.zeros((P, 2, 34), dtype=np.float32)
        sp = np.zeros((3, 2, P), dtype=np.float32)
        for t in range(2):
            m_t = m[t * P : (t + 1) * P]
            nb[0:DH, t, 0] = m_t[0:DH]
            nb[DH:P, t, 1] = m_t[DH:P]
            nb[0:DH, t, 32] = 1.0
            nb[DH:P, t, 33] = 1.0
            coef = -2.0 * ALPHA / SCALE  # -0.0625
            sp[0, t, 0:DH] = coef * m_t[0:DH]
            sp[1, t, DH:P] = coef * m_t[DH:P]
            sp[2, t, :] = 1.0 / SCALE
        in_maps.append(
            {
                "xT": np.ascontiguousarray(x[b].T).astype(NPC),
                "wqT": np.ascontiguousarray(Wq[sl, :].T).astype(NPC),
                "wkT": np.ascontiguousarray(Wk[sl, :].T).astype(NPC),
                "wvT": np.ascontiguousarray(Wv[sl, :].T).astype(NPC),
                "woT": np.ascontiguousarray(Wo[:, sl].T).astype(NPC),
                "normblk": nb.astype(NPC),
                "sprime": sp.astype(NPC),
                "maskT": mt.astype(NPC),
            }
        )
    return in_maps


def kernel(x, Wq, Wk, Wv, Wo, timelike_mask, attn_mask, _trace=False):
    global _NC
    if _NC is None:
        _NC = _build_program()
    nc = _NC

    x = np.asarray(x, dtype=np.float32)
    Wq, Wk, Wv, Wo = (np.asarray(w, dtype=np.float32) for w in (Wq, Wk, Wv, Wo))
    am = np.asarray(attn_mask, dtype=np.float32).reshape(L, L)
    causal = np.tril(np.ones((L, L), dtype=bool))
    assert np.array_equal(am, np.where(causal, 0.0, -1e9).astype(np.float32)), (
        "kernel hardcodes a causal additive mask"
    )

    in_maps = _host_inputs(x, Wq, Wk, Wv, Wo, timelike_mask)
    res = run_bass_kernel_spmd(
        nc, in_maps, core_ids=list(range(N_CORES)), trace=_trace
    )
    outp = np.stack(
        [
            sum(
                res.results[b * HPC + g]["out"].astype(np.float32)
                for g in range(HPC)
            )
            for b in range(B)
        ]
    )
    kernel.last_results = res
    return outp
